# revision 18
# baseline (speedup 1.0000x reference)
"""Trainium2 Bass kernel for a 3-layer GCN (CityAgglomerationGNN).

Strategy (graph/data parallel over 8 NeuronCores):
  - Nodes are ranked by in-degree (desc) and dealt round-robin to cores, so
    every core owns npc = N/8 nodes with matching degree profiles. Within a
    core, nodes are tiled 128-at-a-time.
  - GCN normalization dinv_i*dinv_j is folded into per-partition scalar
    multiplies: table rows are written as dinv*h, aggregates scaled by dinv.
  - Per layer: each core computes its table block and the blocks are
    AllGathered into a replicated fp16 node-feature table in HBM.
  - Aggregation: per dst-tile (128 nodes), in-edge messages are fetched with
    dma_gather (int16 indices; the table is addressed in <=32K-row chunks)
    and reduced into a per-tile PSUM bank via one-hot matmuls. The one-hot S
    matrices are graph-static, built on the host, and streamed from DRAM.
    The self-loop term is applied as an identity matmul against the core's
    own (local) table block, so self-edges are never gathered.
  - Post chain runs on ScalarE: x = (agg)*dinv -> PE transpose -> relu(x + b)
    with the bias per-partition on the transposed side -> matmul with the
    next layer's weights -> *dinv -> next table block.
  - SPMD: one Bass program for all 8 cores; per-(tile,chunk) segment lengths
    are equalized to the max over cores (pad indices gather row 0; their S
    column is zero).
"""

import math
import os
import numpy as np

P = 128
NCORES = 8
GW = 4          # dst tiles per gather window

LAST_RESULT = None  # stash of BassKernelResults for test harness


# ----------------------------------------------------------------------------
# host-side graph preprocessing
# ----------------------------------------------------------------------------

def _host_prep(x, edge_index):
    N = x.shape[0]
    E = edge_index.shape[1]
    assert N % NCORES == 0
    npc = N // NCORES
    NT = (npc + P - 1) // P
    NPCP = NT * P
    NPAD = NCORES * NPCP

    src = np.asarray(edge_index[0], np.int64)
    dst = np.asarray(edge_index[1], np.int64)
    deg = np.bincount(dst, minlength=N).astype(np.float32) + 1.0
    dinv = (1.0 / np.sqrt(deg.astype(np.float64))).astype(np.float32)

    order = np.argsort(-deg, kind="stable")
    ranks = np.empty(N, np.int64)
    ranks[order] = np.arange(N)
    core_of = ranks % NCORES
    slot_of = ranks // NCORES
    split_ag = (NT % 2 == 0)
    if split_ag:
        HS = NPCP // 2                       # slots per half
        half = slot_of // HS
        newid = half * (NCORES * HS) + core_of * HS + slot_of % HS
    else:
        newid = core_of * NPCP + slot_of

    NCH = 1
    for c in (1, 2, 4, 8):
        NCH = c
        if NPAD // c <= 32512:
            break
    if split_ag:
        NCH = max(NCH, 2)
    CH = NPAD // NCH

    es = newid[src]
    ed = newid[dst]

    if split_ag:
        HS = NPCP // 2
        h_e = ed // (NCORES * HS)
        r_e = ed % (NCORES * HS)
        c_e = r_e // HS
        l_e = h_e * HS + r_e % HS
    else:
        c_e = ed // NPCP
        l_e = ed % NPCP
    t_e = l_e // P
    p_e = l_e % P
    m_e = es // CH
    key = (c_e * NT + t_e) * NCH + m_e
    cnt = np.bincount(key, minlength=NCORES * NT * NCH).reshape(NCORES, NT, NCH)
    L = cnt.max(axis=0)                        # [NT, NCH] segment lengths

    eorder = np.argsort(key, kind="stable")
    sidx = (es - m_e * CH)[eorder].astype(np.int16)   # in-chunk row index
    spv = p_e[eorder].astype(np.int64)                # dst slot within tile
    offs = np.zeros(NCORES * NT * NCH + 1, np.int64)
    offs[1:] = np.cumsum(cnt.reshape(-1))

    windows = [(w0, min(w0 + GW, NT)) for w0 in range(0, NT, GW)]

    calls = []             # per (window, chunk): dict(m, G, icol, cols, window)
    core_idx = [[] for _ in range(NCORES)]
    sm_r = [[] for _ in range(NCORES)]
    sm_c = [[] for _ in range(NCORES)]
    window_mms = {}        # (w0,w1) -> {t: [(call_index, g, mmcol), ...]}
    nmm = 0

    for (w0, w1) in windows:
        tiles = list(range(w0, w1))
        wmms = {t: [] for t in tiles}
        window_mms[(w0, w1)] = wmms
        for m in range(NCH):
            # per-core true segment lengths and call positions
            segs = np.zeros((NCORES, len(tiles)), np.int64)
            for ti, t in enumerate(tiles):
                for c in range(NCORES):
                    k = (c * NT + t) * NCH + m
                    segs[c, ti] = offs[k + 1] - offs[k]
            tot = segs.sum(axis=1)
            clen = int(((tot.max() + 15) // 16) * 16)
            if clen == 0:
                continue
            G = (clen + P - 1) // P
            starts = np.zeros((NCORES, len(tiles) + 1), np.int64)
            starts[:, 1:] = np.cumsum(segs, axis=1)

            call_index = len(calls)
            calls.append(dict(m=m, G=G, cols=clen // 16, clen=clen,
                              window=(w0, w1)))

            for ti, t in enumerate(tiles):
                for c in range(NCORES):
                    k = (c * NT + t) * NCH + m
                    n_c = int(segs[c, ti])
                    if n_c == 0:
                        continue
                    pos = int(starts[c, ti]) + np.arange(n_c)
                    core_idx[c].append((call_index, pos,
                                        sidx[offs[k]:offs[k + 1]]))

            # group -> union of tile spans over cores -> matmuls
            for g in range(G):
                glo, ghi = g * P, (g + 1) * P
                for ti, t in enumerate(tiles):
                    hit = False
                    for c in range(NCORES):
                        a, b = int(starts[c, ti]), int(starts[c, ti + 1])
                        if max(a, glo) < min(b, ghi):
                            hit = True
                            break
                    if not hit:
                        continue
                    wmms[t].append((call_index, g, nmm))
                    for c in range(NCORES):
                        k = (c * NT + t) * NCH + m
                        sa = int(offs[k])
                        a, b = int(starts[c, ti]), int(starts[c, ti + 1])
                        lo, hi = max(a, glo), min(b, ghi)
                        if lo >= hi:
                            continue
                        rows = np.arange(lo, hi) - glo
                        cols = nmm * P + spv[sa + (lo - a): sa + (hi - a)]
                        sm_r[c].append(rows)
                        sm_c[c].append(cols)
                    nmm += 1

    icol = 0
    for cl in calls:
        cl["icol"] = icol
        icol += cl["cols"]

    idxs = np.zeros((NCORES, P, icol), np.int16)
    for c in range(NCORES):
        flat = np.zeros(icol * 16, np.int16)
        for call_index, pos, vals in core_idx[c]:
            flat16 = calls[call_index]["icol"] * 16
            flat[flat16 + pos] = vals
        for cl in calls:
            seg = flat[cl["icol"] * 16:(cl["icol"] + cl["cols"]) * 16]
            idxs[c, :16, cl["icol"]:cl["icol"] + cl["cols"]] = \
                seg.reshape(cl["cols"], 16).T
        idxs[c] = np.tile(idxs[c, :16], (8, 1))

    smat = np.zeros((NCORES, P, nmm * P), np.float16)
    for c in range(NCORES):
        if sm_r[c]:
            rr = np.concatenate(sm_r[c])
            cc = np.concatenate(sm_c[c])
            smat[c, rr, cc] = 1.0

    dinv_t = np.zeros((NCORES, P, NT), np.float32)
    loc = slot_of                        # local slot within core
    dinv_t[core_of, loc % P, loc // P] = dinv

    meta = dict(N=N, E=E, npc=npc, NT=NT, NPCP=NPCP, NPAD=NPAD, NCH=NCH, CH=CH,
                windows=windows, calls=calls, window_mms=window_mms, nmm=nmm,
                icols=icol, core_of=core_of, slot_of=slot_of, newid=newid,
                split_ag=split_ag)
    return meta, idxs, smat, dinv_t


# ----------------------------------------------------------------------------
# bass program
# ----------------------------------------------------------------------------

def _build_program(meta, DIN, DH, trace_sim=False):
    import concourse.bass as bass
    import concourse.bacc as bacc
    import concourse.tile as tile
    import concourse.mybir as mybir
    from concourse.masks import make_identity

    f16 = mybir.dt.float16
    f32 = mybir.dt.float32
    i16 = mybir.dt.int16
    Relu = mybir.ActivationFunctionType.Relu
    Copy = mybir.ActivationFunctionType.Copy

    NT, NPCP, NPAD = meta["NT"], meta["NPCP"], meta["NPAD"]
    NCH, CH = meta["NCH"], meta["CH"]
    KD = DIN // P
    calls = meta["calls"]
    windows = meta["windows"]
    window_mms = meta["window_mms"]
    Gmax = max(cl["G"] for cl in calls)
    mms_per_call = {}
    for (w0, w1), wmms in window_mms.items():
        for t, lst in wmms.items():
            for (ci, g, col) in lst:
                mms_per_call.setdefault(ci, []).append(col)

    nc = bacc.Bacc("TRN2", target_bir_lowering=False, debug=False,
                   num_devices=NCORES)

    xT = nc.declare_dram_parameter("xT", [P, KD * NPCP], f16, isOutput=False)
    w1 = nc.declare_dram_parameter("w1", [P, KD * DH], f16, isOutput=False)
    w2 = nc.declare_dram_parameter("w2", [P, DH], f16, isOutput=False)
    w3 = nc.declare_dram_parameter("w3", [P, DH], f16, isOutput=False)
    wc = nc.declare_dram_parameter("wc", [P, 1], f16, isOutput=False)
    bias_p = nc.declare_dram_parameter("biases", [P, 4], f32, isOutput=False)
    dinv_p = nc.declare_dram_parameter("dinv", [P, NT], f32, isOutput=False)
    idxs_p = nc.declare_dram_parameter("idxs", [P, meta["icols"]], i16, isOutput=False)
    smat_p = nc.declare_dram_parameter("smat", [P, meta["nmm"] * P], f16, isOutput=False)
    out_p = nc.declare_dram_parameter("out", [NPCP, 1], f32, isOutput=True)

    with tile.TileContext(nc, trace_sim=trace_sim) as tc:
        with tc.tile_pool(name="const", bufs=1) as cpool, \
             tc.tile_pool(name="dram", bufs=1, space="DRAM") as dpool, \
             tc.tile_pool(name="psum_w", bufs=2, space="PSUM") as wpsp, \
             tc.tile_pool(name="psum_t", bufs=2, space="PSUM") as tpsp, \
             tc.tile_pool(name="psum_a", bufs=4, space="PSUM") as apsp, \
             tc.tile_pool(name="gb", bufs=6) as gpool, \
             tc.tile_pool(name="sm", bufs=3) as spool, \
             tc.tile_pool(name="post", bufs=3) as ppool:

            w1s = cpool.tile([P, KD * DH], f16)
            w2s = cpool.tile([P, DH], f16)
            w3s = cpool.tile([P, DH], f16)
            wcs = cpool.tile([P, 1], f16)
            biases = cpool.tile([P, 4], f32)
            dinvs = cpool.tile([P, NT], f32)
            idxss = cpool.tile([P, meta["icols"]], i16)
            ident = cpool.tile([P, P], f16)
            taba = cpool.tile([P, NT * DH], f16)
            tabb = cpool.tile([P, NT * DH], f16)
            outb = cpool.tile([P, NT], f32)

            for sbuf_t, dram_t in ((w1s, w1), (w2s, w2), (w3s, w3), (wcs, wc),
                                   (biases, bias_p), (dinvs, dinv_p),
                                   (idxss, idxs_p)):
                nc.sync.dma_start(out=sbuf_t[:], in_=dram_t[:])
            make_identity(nc, ident[:])
            with tc.tile_pool(name="warm", bufs=1) as wpool:
                dummy = wpool.tile([P, P], f16)
                nc.gpsimd.dma_gather(
                    out_ap=dummy[:].rearrange("p (g d) -> p g d", g=1),
                    in_ap=smat_p[:].rearrange("p (n d) -> (p n) d", d=P),
                    idxs_ap=idxss[:, :8],
                    num_idxs=P, num_idxs_reg=P, elem_size=DH,
                    single_packet=False)
            for i in range(6):
                z = gpool.tile([P, Gmax * P], f16, tag="gbuf", name=f"z{i}")
                nc.vector.memset(z[:], 0.0)

            split_ag = meta["split_ag"]
            NH = 2 if split_ag else 1
            HS_ROWS = NPCP // NH
            HT_ROWS = NPAD // NH
            agins = {}
            tbls = {}
            for ln in (1, 2, 3):
                agins[ln] = [dpool.tile([HS_ROWS, DH], f16, name=f"agin{ln}_{h}")
                             for h in range(NH)]
                tbls[ln] = [dpool.tile([HT_ROWS, DH], f16, addr_space="Shared",
                                       name=f"tbl{ln}_{h}")
                            for h in range(NH)]

            # ---------------- phase 1: T1 = dinv * (X @ W1) ----------------
            XSL = 14
            with tc.tile_pool(name="xt", bufs=3) as xpool:
                for t0x in range(0, NT, XSL):
                    t1x = min(t0x + XSL, NT)
                    nsl = t1x - t0x
                    xts = xpool.tile([P, KD * XSL * P], f16, tag="xts",
                                     name=f"x{t0x}")
                    nc.sync.dma_start(
                        out=xts[:, :KD * nsl * P].rearrange(
                            "p (k q) -> p k q", k=KD),
                        in_=xT[:].rearrange("p (k n) -> p k n", k=KD)
                             [:, :, t0x * P:t1x * P])
                    for t in range(t0x, t1x):
                        ps = wpsp.tile([P, DH], f32, tag="wps", name=f"d{t}")
                        for k in range(KD):
                            nc.tensor.matmul(
                                out=ps[:],
                                lhsT=xts[:, (k * nsl + (t - t0x)) * P:
                                         (k * nsl + (t - t0x) + 1) * P],
                                rhs=w1s[:, k * DH:(k + 1) * DH],
                                start=(k == 0), stop=(k == KD - 1),
                                skip_group_check=True)
                        nc.scalar.mul(out=taba[:, t * DH:(t + 1) * DH],
                                      in_=ps[:], mul=dinvs[:, t:t + 1])

            NTH = NT // NH

            def table_store_and_ag(tab, ln):
                for h in range(NH):
                    agin, tbl = agins[ln][h], tbls[ln][h]
                    nc.sync.dma_start(
                        out=agin[:].rearrange("(t p) d -> p t d", p=P),
                        in_=tab[:, h * NTH * DH:(h + 1) * NTH * DH]
                            .rearrange("p (t d) -> p t d", d=DH))
                    nc.gpsimd.collective_compute(
                        "AllGather", mybir.AluOpType.bypass,
                        ins=[agin.opt()], outs=[tbl.opt()],
                        replica_groups=[list(range(NCORES))])

            table_store_and_ag(taba, 1)

            layer_cfg = [
                (1, taba, 0, w2s, tabb, 2),
                (2, tabb, 1, w3s, taba, 3),
                (3, taba, 2, None, None, None),
            ]

            nlayers = int(os.environ.get("GCN_LAYERS", "3"))
            for li, (tln, tprev, bi, wnext, tnext, nextln) in \
                    enumerate(layer_cfg[:nlayers]):
                for (w0, w1_) in windows:
                    wcalls = [(ci, cl) for ci, cl in enumerate(calls)
                              if cl["window"] == (w0, w1_)]
                    gbufs = {}
                    sbufs = {}
                    for ci, cl in wcalls:
                        G = cl["G"]
                        gb = gpool.tile([P, Gmax * P], f16, tag="gbuf",
                                        name=f"gb{li}_{ci}")
                        m0 = cl["m"] * CH
                        hh = m0 // HT_ROWS
                        nc.gpsimd.dma_gather(
                            out_ap=gb[:, :G * P].rearrange("p (g d) -> p g d", g=G),
                            in_ap=tbls[tln][hh][m0 - hh * HT_ROWS:
                                                m0 - hh * HT_ROWS + CH, :],
                            idxs_ap=idxss[:, cl["icol"]:cl["icol"] + cl["cols"]],
                            num_idxs=cl["clen"], num_idxs_reg=cl["clen"],
                            elem_size=DH, single_packet=False)
                        gbufs[ci] = gb
                        mmcols = mms_per_call.get(ci, [])
                        if mmcols:
                            c0, c1 = min(mmcols), max(mmcols) + 1
                            sb = spool.tile([P, (c1 - c0) * P], f16, tag="smat",
                                            name=f"sm{li}_{ci}")
                            nc.sync.dma_start(
                                out=sb[:],
                                in_=smat_p[:, c0 * P:c1 * P])
                            sbufs[ci] = (sb, c0)

                    wmms = window_mms[(w0, w1_)]
                    for t in range(w0, w1_):
                        aps = apsp.tile([P, P], f32, tag="agg", name=f"ap{li}_{t}")
                        # self-loop term: identity matmul on own table rows
                        nc.tensor.matmul(
                            out=aps[:], lhsT=ident[:],
                            rhs=tprev[:, t * DH:(t + 1) * DH],
                            start=True, stop=False, skip_group_check=True)
                        lst = wmms[t]
                        for j, (ci, g, col) in enumerate(lst):
                            sb, c0 = sbufs[ci]
                            nc.tensor.matmul(
                                out=aps[:],
                                lhsT=sb[:, (col - c0) * P:(col - c0 + 1) * P],
                                rhs=gbufs[ci][:, g * P:(g + 1) * P],
                                start=False, stop=(j == len(lst) - 1),
                                skip_group_check=True)
                        # ---- post ----
                        tmp = ppool.tile([P, DH], f16, tag="tmp", name=f"tp{li}_{t}")
                        nc.scalar.mul(out=tmp[:], in_=aps[:],
                                      mul=dinvs[:, t:t + 1])
                        tps = tpsp.tile([P, P], f16, tag="tps", name=f"tt{li}_{t}")
                        nc.tensor.transpose(out=tps[:], in_=tmp[:], identity=ident[:])
                        rt = ppool.tile([P, P], f16, tag="rt", name=f"rt{li}_{t}")
                        nc.scalar.activation(out=rt[:], in_=tps[:], func=Relu,
                                             bias=biases[:, bi:bi + 1], scale=1.0)
                        if wnext is not None:
                            wp = wpsp.tile([P, DH], f32, tag="wps", name=f"wp{li}_{t}")
                            nc.tensor.matmul(out=wp[:], lhsT=rt[:], rhs=wnext[:],
                                             start=True, stop=True,
                                             skip_group_check=True)
                            nc.scalar.mul(out=tnext[:, t * DH:(t + 1) * DH],
                                          in_=wp[:], mul=dinvs[:, t:t + 1])
                        else:
                            wp = wpsp.tile([P, DH], f32, tag="wps", name=f"wo{li}_{t}")
                            nc.tensor.matmul(out=wp[:, :1], lhsT=rt[:], rhs=wcs[:],
                                             start=True, stop=True,
                                             skip_group_check=True)
                            nc.scalar.activation(out=outb[:, t:t + 1],
                                                 in_=wp[:, :1], func=Copy,
                                                 bias=float(0.0), scale=1.0)
                            # bc added on host side (scalar)
                if nextln is not None:
                    table_store_and_ag(tnext, nextln)

            nc.sync.dma_start(
                out=out_p[:].rearrange("(t p) o -> p t o", p=P),
                in_=outb[:].unsqueeze(2))

    nc.compile()
    return nc


# ----------------------------------------------------------------------------
# entry point
# ----------------------------------------------------------------------------

def kernel(x, edge_index, W1, b1, W2, b2, W3, b3, Wc, bc):
    global LAST_RESULT
    from concourse.bass_utils import run_bass_kernel_spmd

    x = np.asarray(x)
    N, DIN = x.shape
    DH = np.asarray(W1).shape[1]
    DH2 = np.asarray(W3).shape[1]

    meta, idxs, smat, dinv_t = _host_prep(x, edge_index)
    NT, NPCP = meta["NT"], meta["NPCP"]
    KD = DIN // P

    W3p = np.zeros((DH, DH), np.float32); W3p[:, :DH2] = np.asarray(W3)
    b3p = np.zeros(DH, np.float32); b3p[:DH2] = np.asarray(b3)
    Wcp = np.zeros((DH, 1), np.float32); Wcp[:DH2, :] = np.asarray(Wc)

    biases = np.zeros((P, 4), np.float32)
    biases[:, 0] = np.asarray(b1, np.float32)
    biases[:, 1] = np.asarray(b2, np.float32)
    biases[:, 2] = b3p
    biases[:, 3] = np.float32(np.asarray(bc).reshape(-1)[0])

    w1_sb = np.asarray(W1).reshape(KD, P, DH).transpose(1, 0, 2).reshape(P, KD * DH)
    common = dict(
        w1=w1_sb.astype(np.float16),
        w2=np.asarray(W2).astype(np.float16),
        w3=W3p.astype(np.float16),
        wc=Wcp.astype(np.float16),
        biases=biases,
    )

    core_of, slot_of = meta["core_of"], meta["slot_of"]
    xTc = np.zeros((NCORES, DIN, NPCP), np.float16)
    xf = x.astype(np.float16)
    for c in range(NCORES):
        sel = core_of == c
        xTc[c][:, slot_of[sel]] = xf[sel].T
    xTc = xTc.reshape(NCORES, KD, P, NPCP).transpose(0, 2, 1, 3).reshape(
        NCORES, P, KD * NPCP)

    in_maps = []
    for c in range(NCORES):
        m = dict(common)
        m["xT"] = np.ascontiguousarray(xTc[c])
        m["dinv"] = np.ascontiguousarray(dinv_t[c])
        m["idxs"] = np.ascontiguousarray(idxs[c])
        m["smat"] = np.ascontiguousarray(smat[c])
        in_maps.append(m)

    nc = _build_program(meta, DIN, DH)
    trace = os.environ.get("GCN_TRACE", "") == "1"
    res = run_bass_kernel_spmd(nc, in_maps, list(range(NCORES)), trace=trace)
    LAST_RESULT = res

    bc0 = np.float32(np.asarray(bc).reshape(-1)[0])
    outc = np.stack([res.results[c]["out"][:, 0] for c in range(NCORES)])
    y = (outc[core_of, slot_of] + bc0).astype(np.float32).reshape(N, 1)
    return y


# revision 19
# speedup vs baseline: 1.1202x; 1.1202x over previous
"""Trainium2 Bass kernel for a 3-layer GCN (CityAgglomerationGNN).

Strategy (graph/data parallel over 8 NeuronCores):
  - Nodes are ranked by in-degree (desc) and dealt round-robin to cores, so
    every core owns npc = N/8 nodes with matching degree profiles. Within a
    core, nodes are tiled 128-at-a-time.
  - GCN normalization dinv_i*dinv_j is folded into per-partition scalar
    multiplies: table rows are written as dinv*h, aggregates scaled by dinv.
  - Per layer: each core computes its table block and the blocks are
    AllGathered into a replicated fp16 node-feature table in HBM.
  - Aggregation: per dst-tile (128 nodes), in-edge messages are fetched with
    dma_gather (int16 indices; the table is addressed in <=32K-row chunks)
    and reduced into a per-tile PSUM bank via one-hot matmuls. The one-hot S
    matrices are graph-static, built on the host, and streamed from DRAM.
    The self-loop term is applied as an identity matmul against the core's
    own (local) table block, so self-edges are never gathered.
  - Post chain runs on ScalarE: x = (agg)*dinv -> PE transpose -> relu(x + b)
    with the bias per-partition on the transposed side -> matmul with the
    next layer's weights -> *dinv -> next table block.
  - SPMD: one Bass program for all 8 cores; per-(tile,chunk) segment lengths
    are equalized to the max over cores (pad indices gather row 0; their S
    column is zero).
"""

import math
import os
import numpy as np

P = 128
NCORES = 8
GW = 4          # dst tiles per gather window

LAST_RESULT = None  # stash of BassKernelResults for test harness


# ----------------------------------------------------------------------------
# host-side graph preprocessing
# ----------------------------------------------------------------------------

def _host_prep(x, edge_index):
    N = x.shape[0]
    E = edge_index.shape[1]
    assert N % NCORES == 0
    npc = N // NCORES
    NT = (npc + P - 1) // P
    NPCP = NT * P
    NPAD = NCORES * NPCP

    src = np.asarray(edge_index[0], np.int64)
    dst = np.asarray(edge_index[1], np.int64)
    deg = np.bincount(dst, minlength=N).astype(np.float32) + 1.0
    dinv = (1.0 / np.sqrt(deg.astype(np.float64))).astype(np.float32)

    order = np.argsort(-deg, kind="stable")
    ranks = np.empty(N, np.int64)
    ranks[order] = np.arange(N)
    core_of = ranks % NCORES
    slot_of = ranks // NCORES
    split_ag = False
    if split_ag:
        HS = NPCP // 2                       # slots per half
        half = slot_of // HS
        newid = half * (NCORES * HS) + core_of * HS + slot_of % HS
    else:
        newid = core_of * NPCP + slot_of

    NCH = 1
    for c in (1, 2, 4, 8):
        NCH = c
        if NPAD // c <= 32512:
            break
    if split_ag:
        NCH = max(NCH, 2)
    CH = NPAD // NCH

    es = newid[src]
    ed = newid[dst]

    if split_ag:
        HS = NPCP // 2
        h_e = ed // (NCORES * HS)
        r_e = ed % (NCORES * HS)
        c_e = r_e // HS
        l_e = h_e * HS + r_e % HS
    else:
        c_e = ed // NPCP
        l_e = ed % NPCP
    t_e = l_e // P
    p_e = l_e % P
    m_e = es // CH
    key = (c_e * NT + t_e) * NCH + m_e
    cnt = np.bincount(key, minlength=NCORES * NT * NCH).reshape(NCORES, NT, NCH)
    L = cnt.max(axis=0)                        # [NT, NCH] segment lengths

    eorder = np.argsort(key, kind="stable")
    sidx = (es - m_e * CH)[eorder].astype(np.int16)   # in-chunk row index
    spv = p_e[eorder].astype(np.int64)                # dst slot within tile
    offs = np.zeros(NCORES * NT * NCH + 1, np.int64)
    offs[1:] = np.cumsum(cnt.reshape(-1))

    windows = [(w0, min(w0 + GW, NT)) for w0 in range(0, NT, GW)]

    calls = []             # per (window, chunk): dict(m, G, icol, cols, window)
    core_idx = [[] for _ in range(NCORES)]
    sm_r = [[] for _ in range(NCORES)]
    sm_c = [[] for _ in range(NCORES)]
    window_mms = {}        # (w0,w1) -> {t: [(call_index, g, mmcol), ...]}
    nmm = 0

    for (w0, w1) in windows:
        tiles = list(range(w0, w1))
        wmms = {t: [] for t in tiles}
        window_mms[(w0, w1)] = wmms
        for m in range(NCH):
            # per-core true segment lengths and call positions
            segs = np.zeros((NCORES, len(tiles)), np.int64)
            for ti, t in enumerate(tiles):
                for c in range(NCORES):
                    k = (c * NT + t) * NCH + m
                    segs[c, ti] = offs[k + 1] - offs[k]
            tot = segs.sum(axis=1)
            clen = int(((tot.max() + 15) // 16) * 16)
            if clen == 0:
                continue
            G = (clen + P - 1) // P
            starts = np.zeros((NCORES, len(tiles) + 1), np.int64)
            starts[:, 1:] = np.cumsum(segs, axis=1)

            call_index = len(calls)
            calls.append(dict(m=m, G=G, cols=clen // 16, clen=clen,
                              window=(w0, w1)))

            for ti, t in enumerate(tiles):
                for c in range(NCORES):
                    k = (c * NT + t) * NCH + m
                    n_c = int(segs[c, ti])
                    if n_c == 0:
                        continue
                    pos = int(starts[c, ti]) + np.arange(n_c)
                    core_idx[c].append((call_index, pos,
                                        sidx[offs[k]:offs[k + 1]]))

            # group -> union of tile spans over cores -> matmuls
            for g in range(G):
                glo, ghi = g * P, (g + 1) * P
                for ti, t in enumerate(tiles):
                    hit = False
                    for c in range(NCORES):
                        a, b = int(starts[c, ti]), int(starts[c, ti + 1])
                        if max(a, glo) < min(b, ghi):
                            hit = True
                            break
                    if not hit:
                        continue
                    wmms[t].append((call_index, g, nmm))
                    for c in range(NCORES):
                        k = (c * NT + t) * NCH + m
                        sa = int(offs[k])
                        a, b = int(starts[c, ti]), int(starts[c, ti + 1])
                        lo, hi = max(a, glo), min(b, ghi)
                        if lo >= hi:
                            continue
                        rows = np.arange(lo, hi) - glo
                        cols = nmm * P + spv[sa + (lo - a): sa + (hi - a)]
                        sm_r[c].append(rows)
                        sm_c[c].append(cols)
                    nmm += 1

    icol = 0
    for cl in calls:
        cl["icol"] = icol
        icol += cl["cols"]

    idxs = np.zeros((NCORES, P, icol), np.int16)
    for c in range(NCORES):
        flat = np.zeros(icol * 16, np.int16)
        for call_index, pos, vals in core_idx[c]:
            flat16 = calls[call_index]["icol"] * 16
            flat[flat16 + pos] = vals
        for cl in calls:
            seg = flat[cl["icol"] * 16:(cl["icol"] + cl["cols"]) * 16]
            idxs[c, :16, cl["icol"]:cl["icol"] + cl["cols"]] = \
                seg.reshape(cl["cols"], 16).T
        idxs[c] = np.tile(idxs[c, :16], (8, 1))

    smat = np.zeros((NCORES, P, nmm * P), np.float16)
    for c in range(NCORES):
        if sm_r[c]:
            rr = np.concatenate(sm_r[c])
            cc = np.concatenate(sm_c[c])
            smat[c, rr, cc] = 1.0

    dinv_t = np.zeros((NCORES, P, NT), np.float32)
    loc = slot_of                        # local slot within core
    dinv_t[core_of, loc % P, loc // P] = dinv

    meta = dict(N=N, E=E, npc=npc, NT=NT, NPCP=NPCP, NPAD=NPAD, NCH=NCH, CH=CH,
                windows=windows, calls=calls, window_mms=window_mms, nmm=nmm,
                icols=icol, core_of=core_of, slot_of=slot_of, newid=newid,
                split_ag=split_ag)
    return meta, idxs, smat, dinv_t


# ----------------------------------------------------------------------------
# bass program
# ----------------------------------------------------------------------------

def _build_program(meta, DIN, DH, trace_sim=False):
    import concourse.bass as bass
    import concourse.bacc as bacc
    import concourse.tile as tile
    import concourse.mybir as mybir
    from concourse.masks import make_identity

    f16 = mybir.dt.float16
    f32 = mybir.dt.float32
    i16 = mybir.dt.int16
    Relu = mybir.ActivationFunctionType.Relu
    Copy = mybir.ActivationFunctionType.Copy

    NT, NPCP, NPAD = meta["NT"], meta["NPCP"], meta["NPAD"]
    NCH, CH = meta["NCH"], meta["CH"]
    KD = DIN // P
    calls = meta["calls"]
    windows = meta["windows"]
    window_mms = meta["window_mms"]
    Gmax = max(cl["G"] for cl in calls)
    mms_per_call = {}
    for (w0, w1), wmms in window_mms.items():
        for t, lst in wmms.items():
            for (ci, g, col) in lst:
                mms_per_call.setdefault(ci, []).append(col)

    nc = bacc.Bacc("TRN2", target_bir_lowering=False, debug=False,
                   num_devices=NCORES)

    xT = nc.declare_dram_parameter("xT", [P, KD * NPCP], f16, isOutput=False)
    w1 = nc.declare_dram_parameter("w1", [P, KD * DH], f16, isOutput=False)
    w2 = nc.declare_dram_parameter("w2", [P, DH], f16, isOutput=False)
    w3 = nc.declare_dram_parameter("w3", [P, DH], f16, isOutput=False)
    wc = nc.declare_dram_parameter("wc", [P, 1], f16, isOutput=False)
    bias_p = nc.declare_dram_parameter("biases", [P, 4], f32, isOutput=False)
    dinv_p = nc.declare_dram_parameter("dinv", [P, NT], f32, isOutput=False)
    idxs_p = nc.declare_dram_parameter("idxs", [P, meta["icols"]], i16, isOutput=False)
    smat_p = nc.declare_dram_parameter("smat", [P, meta["nmm"] * P], f16, isOutput=False)
    out_p = nc.declare_dram_parameter("out", [NPCP, 1], f32, isOutput=True)

    with tile.TileContext(nc, trace_sim=trace_sim) as tc:
        with tc.tile_pool(name="const", bufs=1) as cpool, \
             tc.tile_pool(name="dram", bufs=1, space="DRAM") as dpool, \
             tc.tile_pool(name="psum_w", bufs=2, space="PSUM") as wpsp, \
             tc.tile_pool(name="psum_t", bufs=2, space="PSUM") as tpsp, \
             tc.tile_pool(name="psum_a", bufs=4, space="PSUM") as apsp, \
             tc.tile_pool(name="gb", bufs=6) as gpool, \
             tc.tile_pool(name="sm", bufs=3) as spool, \
             tc.tile_pool(name="post", bufs=3) as ppool:

            w1s = cpool.tile([P, KD * DH], f16)
            w2s = cpool.tile([P, DH], f16)
            w3s = cpool.tile([P, DH], f16)
            wcs = cpool.tile([P, 1], f16)
            biases = cpool.tile([P, 4], f32)
            dinvs = cpool.tile([P, NT], f32)
            idxss = cpool.tile([P, meta["icols"]], i16)
            ident = cpool.tile([P, P], f16)
            taba = cpool.tile([P, NT * DH], f16)
            tabb = cpool.tile([P, NT * DH], f16)
            outb = cpool.tile([P, NT], f32)

            for sbuf_t, dram_t in ((w1s, w1), (w2s, w2), (w3s, w3), (wcs, wc),
                                   (biases, bias_p), (dinvs, dinv_p),
                                   (idxss, idxs_p)):
                nc.sync.dma_start(out=sbuf_t[:], in_=dram_t[:])
            make_identity(nc, ident[:])
            with tc.tile_pool(name="warm", bufs=1) as wpool:
                dummy = wpool.tile([P, P], f16)
                nc.gpsimd.dma_gather(
                    out_ap=dummy[:].rearrange("p (g d) -> p g d", g=1),
                    in_ap=smat_p[:].rearrange("p (n d) -> (p n) d", d=P),
                    idxs_ap=idxss[:, :8],
                    num_idxs=P, num_idxs_reg=P, elem_size=DH,
                    single_packet=False)
            for i in range(6):
                z = gpool.tile([P, Gmax * P], f16, tag="gbuf", name=f"z{i}")
                nc.vector.memset(z[:], 0.0)

            split_ag = meta["split_ag"]
            NH = 2 if split_ag else 1
            HS_ROWS = NPCP // NH
            HT_ROWS = NPAD // NH
            agins = {}
            tbls = {}
            for ln in (1, 2, 3):
                agins[ln] = [dpool.tile([HS_ROWS, DH], f16, name=f"agin{ln}_{h}")
                             for h in range(NH)]
                tbls[ln] = [dpool.tile([HT_ROWS, DH], f16, addr_space="Shared",
                                       name=f"tbl{ln}_{h}")
                            for h in range(NH)]

            # ---------------- phase 1: T1 = dinv * (X @ W1) ----------------
            XSL = 14
            with tc.tile_pool(name="xt", bufs=3) as xpool:
                for t0x in range(0, NT, XSL):
                    t1x = min(t0x + XSL, NT)
                    nsl = t1x - t0x
                    xts = xpool.tile([P, KD * XSL * P], f16, tag="xts",
                                     name=f"x{t0x}")
                    nc.sync.dma_start(
                        out=xts[:, :KD * nsl * P].rearrange(
                            "p (k q) -> p k q", k=KD),
                        in_=xT[:].rearrange("p (k n) -> p k n", k=KD)
                             [:, :, t0x * P:t1x * P])
                    for t in range(t0x, t1x):
                        ps = wpsp.tile([P, DH], f32, tag="wps", name=f"d{t}")
                        for k in range(KD):
                            nc.tensor.matmul(
                                out=ps[:],
                                lhsT=xts[:, (k * nsl + (t - t0x)) * P:
                                         (k * nsl + (t - t0x) + 1) * P],
                                rhs=w1s[:, k * DH:(k + 1) * DH],
                                start=(k == 0), stop=(k == KD - 1),
                                skip_group_check=True)
                        nc.scalar.mul(out=taba[:, t * DH:(t + 1) * DH],
                                      in_=ps[:], mul=dinvs[:, t:t + 1])

            NTH = NT // NH

            def table_store_and_ag(tab, ln):
                for h in range(NH):
                    agin, tbl = agins[ln][h], tbls[ln][h]
                    nc.sync.dma_start(
                        out=agin[:].rearrange("(t p) d -> p t d", p=P),
                        in_=tab[:, h * NTH * DH:(h + 1) * NTH * DH]
                            .rearrange("p (t d) -> p t d", d=DH))
                    nc.gpsimd.collective_compute(
                        "AllGather", mybir.AluOpType.bypass,
                        ins=[agin.opt()], outs=[tbl.opt()],
                        replica_groups=[list(range(NCORES))])

            table_store_and_ag(taba, 1)

            layer_cfg = [
                (1, taba, 0, w2s, tabb, 2),
                (2, tabb, 1, w3s, taba, 3),
                (3, taba, 2, None, None, None),
            ]

            nlayers = int(os.environ.get("GCN_LAYERS", "3"))
            for li, (tln, tprev, bi, wnext, tnext, nextln) in \
                    enumerate(layer_cfg[:nlayers]):
                for (w0, w1_) in windows:
                    wcalls = [(ci, cl) for ci, cl in enumerate(calls)
                              if cl["window"] == (w0, w1_)]
                    gbufs = {}
                    sbufs = {}
                    for ci, cl in wcalls:
                        G = cl["G"]
                        gb = gpool.tile([P, Gmax * P], f16, tag="gbuf",
                                        name=f"gb{li}_{ci}")
                        m0 = cl["m"] * CH
                        hh = m0 // HT_ROWS
                        nc.gpsimd.dma_gather(
                            out_ap=gb[:, :G * P].rearrange("p (g d) -> p g d", g=G),
                            in_ap=tbls[tln][hh][m0 - hh * HT_ROWS:
                                                m0 - hh * HT_ROWS + CH, :],
                            idxs_ap=idxss[:, cl["icol"]:cl["icol"] + cl["cols"]],
                            num_idxs=cl["clen"], num_idxs_reg=cl["clen"],
                            elem_size=DH, single_packet=False)
                        gbufs[ci] = gb
                        mmcols = mms_per_call.get(ci, [])
                        if mmcols:
                            c0, c1 = min(mmcols), max(mmcols) + 1
                            sb = spool.tile([P, (c1 - c0) * P], f16, tag="smat",
                                            name=f"sm{li}_{ci}")
                            nc.sync.dma_start(
                                out=sb[:],
                                in_=smat_p[:, c0 * P:c1 * P])
                            sbufs[ci] = (sb, c0)

                    wmms = window_mms[(w0, w1_)]
                    for t in range(w0, w1_):
                        aps = apsp.tile([P, P], f32, tag="agg", name=f"ap{li}_{t}")
                        # self-loop term: identity matmul on own table rows
                        nc.tensor.matmul(
                            out=aps[:], lhsT=ident[:],
                            rhs=tprev[:, t * DH:(t + 1) * DH],
                            start=True, stop=False, skip_group_check=True)
                        lst = wmms[t]
                        for j, (ci, g, col) in enumerate(lst):
                            sb, c0 = sbufs[ci]
                            nc.tensor.matmul(
                                out=aps[:],
                                lhsT=sb[:, (col - c0) * P:(col - c0 + 1) * P],
                                rhs=gbufs[ci][:, g * P:(g + 1) * P],
                                start=False, stop=(j == len(lst) - 1),
                                skip_group_check=True)
                        # ---- post ----
                        tmp = ppool.tile([P, DH], f16, tag="tmp", name=f"tp{li}_{t}")
                        nc.scalar.mul(out=tmp[:], in_=aps[:],
                                      mul=dinvs[:, t:t + 1])
                        tps = tpsp.tile([P, P], f16, tag="tps", name=f"tt{li}_{t}")
                        nc.tensor.transpose(out=tps[:], in_=tmp[:], identity=ident[:])
                        rt = ppool.tile([P, P], f16, tag="rt", name=f"rt{li}_{t}")
                        nc.scalar.activation(out=rt[:], in_=tps[:], func=Relu,
                                             bias=biases[:, bi:bi + 1], scale=1.0)
                        if wnext is not None:
                            wp = wpsp.tile([P, DH], f32, tag="wps", name=f"wp{li}_{t}")
                            nc.tensor.matmul(out=wp[:], lhsT=rt[:], rhs=wnext[:],
                                             start=True, stop=True,
                                             skip_group_check=True)
                            nc.scalar.mul(out=tnext[:, t * DH:(t + 1) * DH],
                                          in_=wp[:], mul=dinvs[:, t:t + 1])
                        else:
                            wp = wpsp.tile([P, DH], f32, tag="wps", name=f"wo{li}_{t}")
                            nc.tensor.matmul(out=wp[:, :1], lhsT=rt[:], rhs=wcs[:],
                                             start=True, stop=True,
                                             skip_group_check=True)
                            nc.scalar.activation(out=outb[:, t:t + 1],
                                                 in_=wp[:, :1], func=Copy,
                                                 bias=float(0.0), scale=1.0)
                            # bc added on host side (scalar)
                if nextln is not None:
                    table_store_and_ag(tnext, nextln)

            nc.sync.dma_start(
                out=out_p[:].rearrange("(t p) o -> p t o", p=P),
                in_=outb[:].unsqueeze(2))

    nc.compile()
    return nc


# ----------------------------------------------------------------------------
# entry point
# ----------------------------------------------------------------------------

def kernel(x, edge_index, W1, b1, W2, b2, W3, b3, Wc, bc):
    global LAST_RESULT
    from concourse.bass_utils import run_bass_kernel_spmd

    x = np.asarray(x)
    N, DIN = x.shape
    DH = np.asarray(W1).shape[1]
    DH2 = np.asarray(W3).shape[1]

    meta, idxs, smat, dinv_t = _host_prep(x, edge_index)
    NT, NPCP = meta["NT"], meta["NPCP"]
    KD = DIN // P

    W3p = np.zeros((DH, DH), np.float32); W3p[:, :DH2] = np.asarray(W3)
    b3p = np.zeros(DH, np.float32); b3p[:DH2] = np.asarray(b3)
    Wcp = np.zeros((DH, 1), np.float32); Wcp[:DH2, :] = np.asarray(Wc)

    biases = np.zeros((P, 4), np.float32)
    biases[:, 0] = np.asarray(b1, np.float32)
    biases[:, 1] = np.asarray(b2, np.float32)
    biases[:, 2] = b3p
    biases[:, 3] = np.float32(np.asarray(bc).reshape(-1)[0])

    w1_sb = np.asarray(W1).reshape(KD, P, DH).transpose(1, 0, 2).reshape(P, KD * DH)
    common = dict(
        w1=w1_sb.astype(np.float16),
        w2=np.asarray(W2).astype(np.float16),
        w3=W3p.astype(np.float16),
        wc=Wcp.astype(np.float16),
        biases=biases,
    )

    core_of, slot_of = meta["core_of"], meta["slot_of"]
    xTc = np.zeros((NCORES, DIN, NPCP), np.float16)
    xf = x.astype(np.float16)
    for c in range(NCORES):
        sel = core_of == c
        xTc[c][:, slot_of[sel]] = xf[sel].T
    xTc = xTc.reshape(NCORES, KD, P, NPCP).transpose(0, 2, 1, 3).reshape(
        NCORES, P, KD * NPCP)

    in_maps = []
    for c in range(NCORES):
        m = dict(common)
        m["xT"] = np.ascontiguousarray(xTc[c])
        m["dinv"] = np.ascontiguousarray(dinv_t[c])
        m["idxs"] = np.ascontiguousarray(idxs[c])
        m["smat"] = np.ascontiguousarray(smat[c])
        in_maps.append(m)

    nc = _build_program(meta, DIN, DH)
    trace = os.environ.get("GCN_TRACE", "") == "1"
    res = run_bass_kernel_spmd(nc, in_maps, list(range(NCORES)), trace=trace)
    LAST_RESULT = res

    bc0 = np.float32(np.asarray(bc).reshape(-1)[0])
    outc = np.stack([res.results[c]["out"][:, 0] for c in range(NCORES)])
    y = (outc[core_of, slot_of] + bc0).astype(np.float32).reshape(N, 1)
    return y
